# revision 13
# baseline (speedup 1.0000x reference)
"""Trainium2 kernel for nn_Decoder_70781061038948.

Pipeline: poly_roots (companion-matrix eigvals) -> KAN layer (20->1024) ->
KAN layer (1024->1024).

Device strategy (8 NeuronCores, pure data-parallel over the 8192 batch):
  * Eigenvalues are computed on host CPU with the exact same jax/LAPACK call
    the reference uses. LAPACK's eigenvalue ORDER comes from the QR
    deflation dynamics and is not reproducible by any on-device sort
    (verified: all 8192 rows have magnitude-order inversions), and the KAN
    layers are order-sensitive, so the only faithful root source is the same
    LAPACK routine. This step is ~0% of FLOPs and bytes.
  * Everything downstream (B-spline bases + both KAN layers = all of the
    compute/memory traffic) runs on the 8 cores, batch-sharded 1024/core.

Math reformulation for the TensorEngine:
  Uniform-knot cubic B-splines are rewritten through the cumulative basis
      Qt_j(x) = sum_{k>=j} B_k(x) = Q((x - t_j)/h),   B_k = Qt_k - Qt_{k+1}
  where Q is a bounded [0,1] C^2 smoothstep.  Then
      sum_k W_k B_k(x) = sum_{j=0..8} (W_j - W_{j-1}) Qt_j(x)
  so each KAN layer becomes ONE dense matmul over 10 features per input
  (silu + 9 bounded Qt values).  6*Qt_j is computed on the vector engines in
  fp32 as a 3rd finite difference of capped relu-cubes
      phi_r = min(relu(2.5x + 5.5 - r), 11 - r)^3
      6*Qt_j = phi_j - 3 phi_{j+1} + 3 phi_{j+2} - phi_{j+3}
  (one fused custom DVE op per phi_r), and only the bounded results are
  rounded to fp32r (11-bit mantissa) for the full-rate fp32r matmul.
  The 1/6 and the weight differencing are folded into the weights on host
  in float64.  End-to-end error vs the fp32 reference: ~6e-4 L2.
"""

import numpy as np
from math import comb
from contextlib import ExitStack

# ---------------------------------------------------------------- constants
K = 10
B = 8192
CORES = 8
BC = B // CORES            # 1024 batch rows per core
IN0 = 2 * K                # 20
HID = 1024
OUT = 1024
NJ = 9                     # Qt features per input
NSEC = 1 + NJ              # silu + 9 Qt
NR = 12                    # relu-cube shifts r = 0..11
# layer-0 c-layout: 10 sections of 32 rows (20 used, partition starts must be
# 32-aligned) -> 320 rows (3 sbuf tiles)
C0_ROWS = 32 * NSEC        # 320
# layer-1 c-layout: c = i_tile*10 + s, 8 i_tiles -> 80 c-tiles of 128
C1_TILES = 8 * NSEC        # 80

_f32 = np.float32


def _round_f32r(a):
    """Round fp32 -> fp32r (11-bit mantissa, round-to-nearest-even)."""
    a = np.ascontiguousarray(a, np.float32)
    u = a.view(np.uint32).astype(np.uint64)
    drop = np.uint64(12)
    one = np.uint64(1)
    half = np.uint64(1 << 11)
    mask = ~np.uint64((1 << 12) - 1)
    r = (u + half - one + ((u >> drop) & one)) & mask
    return r.astype(np.uint32).view(np.float32)


def _poly_roots_host(x):
    """Exact copy of the reference poly_roots, forced onto CPU jax."""
    import jax
    cpu = jax.devices("cpu")[0]
    with jax.default_device(cpu):
        import jax.numpy as jnp
        xj = jax.device_put(np.asarray(x), cpu)
        coeffs = jax.lax.complex(xj[..., 0], xj[..., 1])
        b = coeffs.shape[0]
        norm = coeffs / coeffs[:, :1]
        c = -jnp.flip(norm[:, 1:], axis=-1)
        C = jnp.broadcast_to(jnp.eye(K, k=-1, dtype=coeffs.dtype), (b, K, K))
        C = C.at[:, :, -1].set(c)
        eigs = jnp.linalg.eigvals(C)
        out = jnp.stack([eigs.real, eigs.imag], axis=-1).reshape(b, 2 * K)
        return np.asarray(out.astype(jnp.float32))


def _fold_weights(bw, sw, sc):
    """[O,I] base + [O,I,8]*[O,I] spline weights -> [O, I, 10] folded:
    col 0 = base weight (silu feature), cols 1..9 = (W_j - W_{j-1})/6
    for the 6*Qt_j features."""
    W = sw.astype(np.float64) * sc.astype(np.float64)[..., None]        # [O,I,8]
    O, I = W.shape[:2]
    Wext = np.zeros((O, I, 10))
    Wext[:, :, 1:9] = W                                                  # Wext[m] = W_{m-1}
    wp = (Wext[:, :, 1:10] - Wext[:, :, 0:9]) / 6.0                      # [O,I,9]
    return np.concatenate([bw.astype(np.float64)[:, :, None], wp], axis=2)  # [O,I,10]


# ---------------------------------------------------------------- custom DVE op
_CAPCUBE = None


def _get_capcube():
    """out = min(relu(in0*imm2 + s0), s1)^3  — one DVE pass."""
    global _CAPCUBE
    if _CAPCUBE is not None:
        return _CAPCUBE
    from concourse.dve_spec import Spec, Src0, C0, C1, C2, minn, relu, sq, lower
    from concourse import dve_ops
    from concourse.dve_uop import DveOpSpec

    name = "KAN_CAPCUBE_ANT"
    if name in dve_ops._SUB_OPCODE_FOR_NAME:
        _CAPCUBE = next(op for op in dve_ops.OPS if op.name == name)
        return _CAPCUBE

    def _ref(in0, in1, s0, s1, imm2):
        m = np.minimum(np.maximum(in0 * imm2 + s0, 0.0), s1)
        return (m * m * m).astype(np.float32)

    m = minn(relu(Src0 * C2 + C0), C1)
    spec = Spec(body=sq(m) * m, reference=_ref)
    shas = {}
    for ver in ("v3", "v4"):
        try:
            s = DveOpSpec(name=name, opcode=1, uops=lower(spec, ver=ver), rd1_en=False)
            shas[ver] = s.sha(ver)
        except Exception:
            pass
    op = dve_ops.DveOp(name, spec, subdim=False, uops_sha=shas)
    dve_ops.OPS.append(op)
    dve_ops.CUSTOM_DVE_SPECS[name] = spec
    dve_ops._SUB_OPCODE_FOR_NAME[name] = dve_ops._CUSTOM_DVE_ROW_BASE + len(dve_ops.OPS) - 1
    _CAPCUBE = op
    return op


# ---------------------------------------------------------------- bass program
_PROGRAM = None


def build_program():
    """Build (and cache) the compiled per-core Bass program."""
    global _PROGRAM
    if _PROGRAM is not None:
        return _PROGRAM

    import concourse.bacc as bacc
    import concourse.bass as bass
    import concourse.tile as tile
    import concourse.mybir as mybir

    F32 = mybir.dt.float32
    F32R = mybir.dt.float32r
    AFT = mybir.ActivationFunctionType
    ALU = mybir.AluOpType
    CAPCUBE = _get_capcube()

    nc = bacc.Bacc("TRN2", target_bir_lowering=False, debug=False)

    rt_d = nc.dram_tensor("rt", [IN0, BC], F32, kind="ExternalInput")
    w0_d = nc.dram_tensor("w0t", [C0_ROWS, HID], F32R, kind="ExternalInput")
    w1_d = nc.dram_tensor("w1t", [C1_TILES, 128, OUT], F32R, kind="ExternalInput")
    out_d = nc.dram_tensor("out", [OUT, BC], F32, kind="ExternalOutput")

    with tile.TileContext(nc) as tc:
        with ExitStack() as ctx:
            const = ctx.enter_context(tc.tile_pool(name="const", bufs=1))
            f0p = ctx.enter_context(tc.tile_pool(name="f0p", bufs=1))
            tmp0p = ctx.enter_context(tc.tile_pool(name="tmp0p", bufs=2))
            hp = ctx.enter_context(tc.tile_pool(name="hp", bufs=8))
            phip = ctx.enter_context(tc.tile_pool(name="phip", bufs=1))
            chp = ctx.enter_context(tc.tile_pool(name="chp", bufs=4))
            qtp = ctx.enter_context(tc.tile_pool(name="qtp", bufs=2))
            silp = ctx.enter_context(tc.tile_pool(name="silp", bufs=3))
            w1p = ctx.enter_context(tc.tile_pool(name="w1p", bufs=4))
            osp = ctx.enter_context(tc.tile_pool(name="osp", bufs=4))
            psp = ctx.enter_context(tc.tile_pool(name="psp", bufs=8, space="PSUM"))

            # ---------------- layer 0 ----------------
            rt_sb = const.tile([IN0, BC], F32)
            nc.sync.dma_start(rt_sb[:], rt_d.ap())

            f0 = [
                f0p.tile([128, BC], F32R, name="f0a"),
                f0p.tile([128, BC], F32R, name="f0b"),
                f0p.tile([64, BC], F32R, name="f0c"),
            ]
            # zero-fill (pad rows must be finite): DVE memset an f32 tile,
            # ACT-copy into the f32r tiles (activation producers round to f32r)
            zt = const.tile([128, BC], F32, name="zt")
            nc.vector.memset(zt[:], 0.0)
            for t in f0:
                p = t.shape[0]
                nc.scalar.copy(t[:], zt[0:p, :])

            # silu section (s=0) -> rows 0..19 of f0[0]
            nc.scalar.activation(f0[0][0:IN0, :], rt_sb[:], AFT.Silu)
            # Qt sections (s=1..9 at row 32*s), b in halves to bound phi size
            for bh in range(2):
                bs = slice(bh * 512, (bh + 1) * 512)
                phi0 = phip.tile([IN0, NR * 512], F32, name="phi")
                for r in range(NR):
                    nc.vector._custom_dve(
                        CAPCUBE,
                        out=phi0[:, r * 512:(r + 1) * 512],
                        in0=rt_sb[:, bs],
                        s0=float(5.5 - r),
                        s1=float(11 - r),
                        imm2=2.5,
                    )
                for j in range(NJ):
                    a = tmp0p.tile([IN0, 512], F32, name="l0a")
                    eng = nc.vector
                    eng.scalar_tensor_tensor(
                        a[:], phi0[:, (j + 1) * 512:(j + 2) * 512], -3.0,
                        phi0[:, j * 512:(j + 1) * 512], ALU.mult, ALU.add)
                    b = tmp0p.tile([IN0, 512], F32, name="l0b")
                    eng.scalar_tensor_tensor(
                        b[:], phi0[:, (j + 2) * 512:(j + 3) * 512], 3.0,
                        a[:], ALU.mult, ALU.add)
                    row = 32 * (1 + j)
                    ti, off = row // 128, row % 128
                    eng.scalar_tensor_tensor(
                        f0[ti][off:off + IN0, bs],
                        phi0[:, (j + 3) * 512:(j + 4) * 512], -1.0,
                        b[:], ALU.mult, ALU.add)

            w0 = [
                const.tile([128, HID], F32R, name="w0a"),
                const.tile([128, HID], F32R, name="w0b"),
                const.tile([64, HID], F32R, name="w0c"),
            ]
            nc.sync.dma_start(w0[0][:], w0_d.ap()[0:128, :])
            nc.sync.dma_start(w0[1][:], w0_d.ap()[128:256, :])
            nc.sync.dma_start(w0[2][:], w0_d.ap()[256:C0_ROWS, :])

            hT = [hp.tile([128, BC], F32, name="hT") for _ in range(8)]
            for bh in range(2):
                bsl = slice(bh * 512, (bh + 1) * 512)
                for o in range(8):
                    ps = psp.tile([128, 512], F32, name="ps")
                    for t in range(3):
                        nc.tensor.matmul(
                            ps[:], w0[t][:, o * 128:(o + 1) * 128], f0[t][:, bsl],
                            start=(t == 0), stop=(t == 2))
                    nc.scalar.copy(hT[o][:, bsl], ps[:])

            # ---------------- layer 1 ----------------
            for ch in range(2):
                bsl = slice(ch * 512, (ch + 1) * 512)
                pss = [psp.tile([128, 512], F32, name="ps") for _ in range(8)]
                for it in range(8):
                    x = hT[it][:, bsl]
                    phi = phip.tile([128, NR * 512], F32, name="phi")
                    for r in range(NR):
                        nc.vector._custom_dve(
                            CAPCUBE,
                            out=phi[:, r * 512:(r + 1) * 512],
                            in0=x,
                            s0=float(5.5 - r),
                            s1=float(11 - r),
                            imm2=2.5,
                        )
                    sil = silp.tile([128, 512], F32R, name="sil")
                    nc.scalar.activation(sil[:], x, AFT.Silu)
                    qt = qtp.tile([128, NJ * 512], F32R, name="qt")
                    for g in range(3):  # j-groups {0-2},{3-5},{6-8}
                        j0 = 3 * g
                        gw = 3 * 512
                        a1 = chp.tile([128, gw], F32, name="ch")
                        nc.vector.scalar_tensor_tensor(
                            a1[:], phi[:, (j0 + 1) * 512:(j0 + 1) * 512 + gw], -3.0,
                            phi[:, j0 * 512:j0 * 512 + gw], ALU.mult, ALU.add)
                        a2 = chp.tile([128, gw], F32, name="ch")
                        nc.vector.scalar_tensor_tensor(
                            a2[:], phi[:, (j0 + 2) * 512:(j0 + 2) * 512 + gw], 3.0,
                            a1[:], ALU.mult, ALU.add)
                        nc.vector.scalar_tensor_tensor(
                            qt[:, j0 * 512:j0 * 512 + gw],
                            phi[:, (j0 + 3) * 512:(j0 + 3) * 512 + gw], -1.0,
                            a2[:], ALU.mult, ALU.add)
                    for s in range(NSEC):
                        c = it * NSEC + s
                        w = w1p.tile([128, OUT], F32R, name="w1")
                        nc.sync.dma_start(w[:], w1_d.ap()[c, :, :])
                        F = sil[:] if s == 0 else qt[:, (s - 1) * 512:s * 512]
                        for o in range(8):
                            nc.tensor.matmul(
                                pss[o][:], w[:, o * 128:(o + 1) * 128], F,
                                start=(c == 0), stop=(c == C1_TILES - 1))
                for o in range(8):
                    st = osp.tile([128, 512], F32, name="ost")
                    nc.scalar.copy(st[:], pss[o][:])
                    nc.sync.dma_start(out_d.ap()[o * 128:(o + 1) * 128, bsl], st[:])

    nc.compile()
    _PROGRAM = nc
    return nc


# ---------------------------------------------------------------- host driver
_INPUT_CACHE = {}


def prepare_inputs(x, bw0, sw0, sc0, bw1, sw1, sc1):
    """Host-side prep: roots + folded/pre-rounded weights + per-core in_maps."""
    roots = _poly_roots_host(np.asarray(x, np.float32))          # [B, 20]
    rootsT = np.ascontiguousarray(roots.T)                        # [20, B]

    W0 = _fold_weights(bw0, sw0, sc0)                             # [1024, 20, 10]
    W1 = _fold_weights(bw1, sw1, sc1)                             # [1024, 1024, 10]

    # layer-0 DRAM layout [320, 1024]: row 32*s + i  -> W0[o, i, s] (pads 0)
    w0t = np.zeros((C0_ROWS, HID), np.float64)
    for s in range(NSEC):
        w0t[32 * s:32 * s + IN0, :] = W0[:, :, s].T               # [20, 1024]
    w0t = _round_f32r(w0t.astype(np.float32))

    # layer-1 DRAM layout [80, 128, 1024]: (c = it*10 + s, p, o) -> W1[o, it*128+p, s]
    w1t = np.empty((C1_TILES, 128, OUT), np.float32)
    for it in range(8):
        blk = W1[:, it * 128:(it + 1) * 128, :]                   # [O, 128, 10]
        for s in range(NSEC):
            w1t[it * NSEC + s] = blk[:, :, s].T.astype(np.float32)
    w1t = _round_f32r(w1t)

    in_maps = []
    for c in range(CORES):
        in_maps.append({
            "rt": np.ascontiguousarray(rootsT[:, c * BC:(c + 1) * BC]),
            "w0t": w0t,
            "w1t": w1t,
        })
    return in_maps


def assemble_output(results):
    """Per-core [OUT, BC] (o, b) outputs -> full [B, OUT]."""
    return np.ascontiguousarray(
        np.concatenate([np.asarray(r["out"]).T for r in results], axis=0)
    ).astype(np.float32)


def kernel(x, bw0, sw0, sc0, bw1, sw1, sc1):
    from concourse.bass_utils import run_bass_kernel_spmd
    nc = build_program()
    in_maps = prepare_inputs(x, bw0, sw0, sc0, bw1, sw1, sc1)
    res = run_bass_kernel_spmd(nc, in_maps, list(range(CORES)))
    return assemble_output(res.results)


# revision 19
# speedup vs baseline: 14163.3275x; 14163.3275x over previous
"""Trainium2 kernel for nn_Decoder_70781061038948.

Pipeline: poly_roots (companion-matrix eigvals) -> KAN layer (20->1024) ->
KAN layer (1024->1024).

Device strategy (8 NeuronCores, pure data-parallel over the 8192 batch):
  * Eigenvalues are computed on host CPU with the exact same jax/LAPACK call
    the reference uses. LAPACK's eigenvalue ORDER comes from the QR
    deflation dynamics and is not reproducible by any on-device sort
    (verified: all 8192 rows have magnitude-order inversions), and the KAN
    layers are order-sensitive, so the only faithful root source is the same
    LAPACK routine. This step is ~0% of FLOPs and bytes.
  * Everything downstream (B-spline bases + both KAN layers = all of the
    compute/memory traffic) runs on the 8 cores, batch-sharded 1024/core.

Math reformulation for the TensorEngine:
  Uniform-knot cubic B-splines are rewritten through the cumulative basis
      Qt_j(x) = sum_{k>=j} B_k(x) = Q((x - t_j)/h),   B_k = Qt_k - Qt_{k+1}
  where Q is a bounded [0,1] C^2 smoothstep.  Then
      sum_k W_k B_k(x) = sum_{j=0..8} (W_j - W_{j-1}) Qt_j(x)
  so each KAN layer becomes ONE dense matmul over 10 features per input
  (silu + 9 bounded Qt values).  6*Qt_j is computed on the vector engines in
  fp32 as a 3rd finite difference of capped relu-cubes
      phi_r = min(relu(2.5x + 5.5 - r), 11 - r)^3
      6*Qt_j = phi_j - 3 phi_{j+1} + 3 phi_{j+2} - phi_{j+3}
  (one fused custom DVE op per phi_r), and only the bounded results are
  rounded to fp32r (11-bit mantissa) for the full-rate fp32r matmul.
  The 1/6 and the weight differencing are folded into the weights on host
  in float64.  End-to-end error vs the fp32 reference: ~6e-4 L2.
"""

import numpy as np
from math import comb
from contextlib import ExitStack

# ---------------------------------------------------------------- constants
K = 10
B = 8192
CORES = 8
BC = B // CORES            # 1024 batch rows per core
IN0 = 2 * K                # 20
HID = 1024
OUT = 1024
NJ = 9                     # Qt features per input
NSEC = 1 + NJ              # silu + 9 Qt
NR = 12                    # relu-cube shifts r = 0..11
# layer-0 c-layout: 10 sections of 32 rows (20 used, partition starts must be
# 32-aligned) -> 320 rows (3 sbuf tiles)
C0_ROWS = 32 * NSEC        # 320
# layer-1 c-layout: c = i_tile*10 + s, 8 i_tiles -> 80 c-tiles of 128
C1_TILES = 8 * NSEC        # 80

_f32 = np.float32


def _round_f32r(a):
    """Round fp32 -> fp32r (11-bit mantissa, round-to-nearest-even)."""
    a = np.ascontiguousarray(a, np.float32)
    u = a.view(np.uint32).astype(np.uint64)
    drop = np.uint64(12)
    one = np.uint64(1)
    half = np.uint64(1 << 11)
    mask = ~np.uint64((1 << 12) - 1)
    r = (u + half - one + ((u >> drop) & one)) & mask
    return r.astype(np.uint32).view(np.float32)


def _poly_roots_host(x):
    """Exact copy of the reference poly_roots, forced onto CPU jax."""
    import jax
    cpu = jax.devices("cpu")[0]
    with jax.default_device(cpu):
        import jax.numpy as jnp
        xj = jax.device_put(np.asarray(x), cpu)
        coeffs = jax.lax.complex(xj[..., 0], xj[..., 1])
        b = coeffs.shape[0]
        norm = coeffs / coeffs[:, :1]
        c = -jnp.flip(norm[:, 1:], axis=-1)
        C = jnp.broadcast_to(jnp.eye(K, k=-1, dtype=coeffs.dtype), (b, K, K))
        C = C.at[:, :, -1].set(c)
        eigs = jnp.linalg.eigvals(C)
        out = jnp.stack([eigs.real, eigs.imag], axis=-1).reshape(b, 2 * K)
        return np.asarray(out.astype(jnp.float32))


def _fold_weights(bw, sw, sc):
    """[O,I] base + [O,I,8]*[O,I] spline weights -> [O, I, 10] folded:
    col 0 = base weight (silu feature), cols 1..9 = (W_j - W_{j-1})/6
    for the 6*Qt_j features."""
    W = sw.astype(np.float64) * sc.astype(np.float64)[..., None]        # [O,I,8]
    O, I = W.shape[:2]
    Wext = np.zeros((O, I, 10))
    Wext[:, :, 1:9] = W                                                  # Wext[m] = W_{m-1}
    wp = (Wext[:, :, 1:10] - Wext[:, :, 0:9]) / 6.0                      # [O,I,9]
    return np.concatenate([bw.astype(np.float64)[:, :, None], wp], axis=2)  # [O,I,10]


# ---------------------------------------------------------------- custom DVE op
_CAPCUBE = None


def _get_capcube():
    """out = min(relu(in0*imm2 + s0), s1)^3  — one DVE pass."""
    global _CAPCUBE
    if _CAPCUBE is not None:
        return _CAPCUBE
    from concourse.dve_spec import Spec, Src0, C0, C1, C2, minn, relu, sq, lower
    from concourse import dve_ops
    from concourse.dve_uop import DveOpSpec

    name = "KAN_CAPCUBE_ANT"
    if name in dve_ops._SUB_OPCODE_FOR_NAME:
        _CAPCUBE = next(op for op in dve_ops.OPS if op.name == name)
        return _CAPCUBE

    def _ref(in0, in1, s0, s1, imm2):
        m = np.minimum(np.maximum(in0 * imm2 + s0, 0.0), s1)
        return (m * m * m).astype(np.float32)

    m = minn(relu(Src0 * C2 + C0), C1)
    spec = Spec(body=sq(m) * m, reference=_ref)
    shas = {}
    for ver in ("v3", "v4"):
        try:
            s = DveOpSpec(name=name, opcode=1, uops=lower(spec, ver=ver), rd1_en=False)
            shas[ver] = s.sha(ver)
        except Exception:
            pass
    op = dve_ops.DveOp(name, spec, subdim=False, uops_sha=shas)
    dve_ops.OPS.append(op)
    dve_ops.CUSTOM_DVE_SPECS[name] = spec
    dve_ops._SUB_OPCODE_FOR_NAME[name] = dve_ops._CUSTOM_DVE_ROW_BASE + len(dve_ops.OPS) - 1
    _CAPCUBE = op
    return op


# ---------------------------------------------------------------- bass program
_PROGRAMS = {}

# phi shifts computed via ACT(relu) + DVE(min) + GPSIMD(sq, cube) instead of
# the fused DVE custom op, to balance engine load (DVE is the bottleneck)
_MOVED_R = (1, 4, 7, 10)


def build_program(iters=1):
    """Build (and cache) the compiled per-core Bass program.  iters>1 wraps
    the whole body in a device-side loop (for timing: amortizes the ~100 ms
    axon tunnel round-trip over many kernel executions)."""
    if iters in _PROGRAMS:
        return _PROGRAMS[iters]

    import concourse.bacc as bacc
    import concourse.bass as bass
    import concourse.tile as tile
    import concourse.mybir as mybir

    F32 = mybir.dt.float32
    F32R = mybir.dt.float32r
    AFT = mybir.ActivationFunctionType
    ALU = mybir.AluOpType
    CAPCUBE = _get_capcube()

    nc = bacc.Bacc("TRN2", target_bir_lowering=False, debug=False)

    rt_d = nc.dram_tensor("rt", [IN0, BC], F32, kind="ExternalInput")
    w0_d = nc.dram_tensor("w0t", [C0_ROWS, HID], F32R, kind="ExternalInput")
    w1_d = nc.dram_tensor("w1t", [C1_TILES, 128, OUT], F32R, kind="ExternalInput")
    out_d = nc.dram_tensor("out", [OUT, BC], F32, kind="ExternalOutput")

    with tile.TileContext(nc) as tc:
        with ExitStack() as ctx:
            const = ctx.enter_context(tc.tile_pool(name="const", bufs=1))
            f0p = ctx.enter_context(tc.tile_pool(name="f0p", bufs=1))
            tmp0p = ctx.enter_context(tc.tile_pool(name="tmp0p", bufs=2))
            hp = ctx.enter_context(tc.tile_pool(name="hp", bufs=8))
            phip = ctx.enter_context(tc.tile_pool(name="phip", bufs=1))
            chp = ctx.enter_context(tc.tile_pool(name="chp", bufs=3))
            qtp = ctx.enter_context(tc.tile_pool(name="qtp", bufs=2))
            silp = ctx.enter_context(tc.tile_pool(name="silp", bufs=2))
            w1p = ctx.enter_context(tc.tile_pool(name="w1p", bufs=3))
            osp = ctx.enter_context(tc.tile_pool(name="osp", bufs=4))
            scrp = ctx.enter_context(tc.tile_pool(name="scrp", bufs=6))
            psp = ctx.enter_context(tc.tile_pool(name="psp", bufs=8, space="PSUM"))

            # per-partition bias constants for the ACT-path relu (5.5 - r)
            biast = const.tile([128, len(_MOVED_R)], F32, name="biast")
            for i, r in enumerate(_MOVED_R):
                nc.vector.memset(biast[:, i:i + 1], float(5.5 - r))

            loop_cm = tc.For_i(0, iters, 1) if iters > 1 else None
            if loop_cm is not None:
                loop_cm.__enter__()

            # ---------------- layer 0 ----------------
            rt_sb = const.tile([IN0, BC], F32)
            nc.sync.dma_start(rt_sb[:], rt_d.ap())

            f0 = [
                f0p.tile([128, BC], F32R, name="f0a"),
                f0p.tile([128, BC], F32R, name="f0b"),
                f0p.tile([64, BC], F32R, name="f0c"),
            ]
            # zero-fill (pad rows must be finite): DVE memset an f32 tile,
            # ACT-copy into the f32r tiles (activation producers round to f32r)
            zt = const.tile([128, BC], F32, name="zt")
            nc.vector.memset(zt[:], 0.0)
            for t in f0:
                p = t.shape[0]
                nc.scalar.copy(t[:], zt[0:p, :])

            # silu section (s=0) -> rows 0..19 of f0[0]
            nc.scalar.activation(f0[0][0:IN0, :], rt_sb[:], AFT.Silu)
            # Qt sections (s=1..9 at row 32*s), b in halves to bound phi size
            for bh in range(2):
                bs = slice(bh * 512, (bh + 1) * 512)
                phi0 = phip.tile([IN0, NR * 512], F32, name="phi")
                for r in range(NR):
                    nc.vector._custom_dve(
                        CAPCUBE,
                        out=phi0[:, r * 512:(r + 1) * 512],
                        in0=rt_sb[:, bs],
                        s0=float(5.5 - r),
                        s1=float(11 - r),
                        imm2=2.5,
                    )
                for j in range(NJ):
                    a = tmp0p.tile([IN0, 512], F32, name="l0a")
                    eng = nc.vector
                    eng.scalar_tensor_tensor(
                        a[:], phi0[:, (j + 1) * 512:(j + 2) * 512], -3.0,
                        phi0[:, j * 512:(j + 1) * 512], ALU.mult, ALU.add)
                    b = tmp0p.tile([IN0, 512], F32, name="l0b")
                    eng.scalar_tensor_tensor(
                        b[:], phi0[:, (j + 2) * 512:(j + 3) * 512], 3.0,
                        a[:], ALU.mult, ALU.add)
                    row = 32 * (1 + j)
                    ti, off = row // 128, row % 128
                    eng.scalar_tensor_tensor(
                        f0[ti][off:off + IN0, bs],
                        phi0[:, (j + 3) * 512:(j + 4) * 512], -1.0,
                        b[:], ALU.mult, ALU.add)

            w0 = [
                const.tile([128, HID], F32R, name="w0a"),
                const.tile([128, HID], F32R, name="w0b"),
                const.tile([64, HID], F32R, name="w0c"),
            ]
            nc.sync.dma_start(w0[0][:], w0_d.ap()[0:128, :])
            nc.sync.dma_start(w0[1][:], w0_d.ap()[128:256, :])
            nc.sync.dma_start(w0[2][:], w0_d.ap()[256:C0_ROWS, :])

            hT = [hp.tile([128, BC], F32, name="hT") for _ in range(8)]
            for bh in range(2):
                bsl = slice(bh * 512, (bh + 1) * 512)
                for o in range(8):
                    ps = psp.tile([128, 512], F32, name="ps")
                    for t in range(3):
                        nc.tensor.matmul(
                            ps[:], w0[t][:, o * 128:(o + 1) * 128], f0[t][:, bsl],
                            start=(t == 0), stop=(t == 2))
                    nc.scalar.copy(hT[o][:, bsl], ps[:])

            # ---------------- layer 1 ----------------
            for ch in range(2):
                bsl = slice(ch * 512, (ch + 1) * 512)
                pss = [psp.tile([128, 512], F32, name="ps") for _ in range(8)]
                for it in range(8):
                    x = hT[it][:, bsl]
                    phi = phip.tile([128, NR * 512], F32, name="phi")
                    v25 = scrp.tile([128, 512], F32, name="v25", bufs=2)
                    nc.vector.tensor_scalar_mul(v25[:], x, 2.5)
                    for r in range(NR):
                        if r in _MOVED_R:
                            # ACT + DVE(min) + GPSIMD path to offload DVE
                            rl = scrp.tile([128, 512], F32, name="scr")
                            bi = _MOVED_R.index(r)
                            nc.scalar.activation(rl[:], v25[:], AFT.Relu,
                                                 bias=biast[:, bi:bi + 1])
                            mn = scrp.tile([128, 512], F32, name="scr")
                            nc.vector.tensor_scalar_min(mn[:], rl[:],
                                                        float(11 - r))
                            sq = scrp.tile([128, 512], F32, name="scr")
                            nc.gpsimd.tensor_mul(sq[:], mn[:], mn[:])
                            nc.gpsimd.tensor_mul(
                                phi[:, r * 512:(r + 1) * 512], sq[:], mn[:])
                        else:
                            nc.vector._custom_dve(
                                CAPCUBE,
                                out=phi[:, r * 512:(r + 1) * 512],
                                in0=x,
                                s0=float(5.5 - r),
                                s1=float(11 - r),
                                imm2=2.5,
                            )
                    sil = silp.tile([128, 512], F32R, name="sil")
                    nc.scalar.activation(sil[:], x, AFT.Silu)
                    qt = qtp.tile([128, NJ * 512], F32R, name="qt")
                    for g in range(3):  # j-groups {0-2},{3-5},{6-8}
                        j0 = 3 * g
                        gw = 3 * 512
                        a1 = chp.tile([128, gw], F32, name="ch")
                        nc.vector.scalar_tensor_tensor(
                            a1[:], phi[:, (j0 + 1) * 512:(j0 + 1) * 512 + gw], -3.0,
                            phi[:, j0 * 512:j0 * 512 + gw], ALU.mult, ALU.add)
                        a2 = chp.tile([128, gw], F32, name="ch")
                        nc.vector.scalar_tensor_tensor(
                            a2[:], phi[:, (j0 + 2) * 512:(j0 + 2) * 512 + gw], 3.0,
                            a1[:], ALU.mult, ALU.add)
                        nc.vector.scalar_tensor_tensor(
                            qt[:, j0 * 512:j0 * 512 + gw],
                            phi[:, (j0 + 3) * 512:(j0 + 3) * 512 + gw], -1.0,
                            a2[:], ALU.mult, ALU.add)
                    for s in range(NSEC):
                        c = it * NSEC + s
                        w = w1p.tile([128, OUT], F32R, name="w1")
                        nc.sync.dma_start(w[:], w1_d.ap()[c, :, :])
                        F = sil[:] if s == 0 else qt[:, (s - 1) * 512:s * 512]
                        for o in range(8):
                            nc.tensor.matmul(
                                pss[o][:], w[:, o * 128:(o + 1) * 128], F,
                                start=(c == 0), stop=(c == C1_TILES - 1))
                for o in range(8):
                    st = osp.tile([128, 512], F32, name="ost")
                    nc.scalar.copy(st[:], pss[o][:])
                    nc.sync.dma_start(out_d.ap()[o * 128:(o + 1) * 128, bsl], st[:])

            if loop_cm is not None:
                loop_cm.__exit__(None, None, None)

    nc.compile()
    _PROGRAMS[iters] = nc
    return nc


# ---------------------------------------------------------------- host driver
_INPUT_CACHE = {}


def prepare_inputs(x, bw0, sw0, sc0, bw1, sw1, sc1):
    """Host-side prep: roots + folded/pre-rounded weights + per-core in_maps."""
    roots = _poly_roots_host(np.asarray(x, np.float32))          # [B, 20]
    rootsT = np.ascontiguousarray(roots.T)                        # [20, B]

    W0 = _fold_weights(bw0, sw0, sc0)                             # [1024, 20, 10]
    W1 = _fold_weights(bw1, sw1, sc1)                             # [1024, 1024, 10]

    # layer-0 DRAM layout [320, 1024]: row 32*s + i  -> W0[o, i, s] (pads 0)
    w0t = np.zeros((C0_ROWS, HID), np.float64)
    for s in range(NSEC):
        w0t[32 * s:32 * s + IN0, :] = W0[:, :, s].T               # [20, 1024]
    w0t = _round_f32r(w0t.astype(np.float32))

    # layer-1 DRAM layout [80, 128, 1024]: (c = it*10 + s, p, o) -> W1[o, it*128+p, s]
    w1t = np.empty((C1_TILES, 128, OUT), np.float32)
    for it in range(8):
        blk = W1[:, it * 128:(it + 1) * 128, :]                   # [O, 128, 10]
        for s in range(NSEC):
            w1t[it * NSEC + s] = blk[:, :, s].T.astype(np.float32)
    w1t = _round_f32r(w1t)

    in_maps = []
    for c in range(CORES):
        in_maps.append({
            "rt": np.ascontiguousarray(rootsT[:, c * BC:(c + 1) * BC]),
            "w0t": w0t,
            "w1t": w1t,
        })
    return in_maps


def assemble_output(results):
    """Per-core [OUT, BC] (o, b) outputs -> full [B, OUT]."""
    return np.ascontiguousarray(
        np.concatenate([np.asarray(r["out"]).T for r in results], axis=0)
    ).astype(np.float32)


def kernel(x, bw0, sw0, sc0, bw1, sw1, sc1):
    from concourse.bass_utils import run_bass_kernel_spmd
    nc = build_program()
    in_maps = prepare_inputs(x, bw0, sw0, sc0, bw1, sw1, sc1)
    res = run_bass_kernel_spmd(nc, in_maps, list(range(CORES)))
    return assemble_output(res.results)


# revision 25
# speedup vs baseline: 18795.2355x; 1.3270x over previous
"""Trainium2 kernel for nn_Decoder_70781061038948.

Pipeline: poly_roots (companion-matrix eigvals) -> KAN layer (20->1024) ->
KAN layer (1024->1024).

Device strategy (8 NeuronCores, pure data-parallel over the 8192 batch):
  * Eigenvalues are computed on host CPU with the exact same jax/LAPACK call
    the reference uses. LAPACK's eigenvalue ORDER comes from the QR
    deflation dynamics and is not reproducible by any on-device sort
    (verified: all 8192 rows have magnitude-order inversions), and the KAN
    layers are order-sensitive, so the only faithful root source is the same
    LAPACK routine. This step is ~0% of FLOPs and bytes.
  * Everything downstream (B-spline bases + both KAN layers = all of the
    compute/memory traffic) runs on the 8 cores, batch-sharded 1024/core.

Math reformulation for the TensorEngine:
  Uniform-knot cubic B-splines are rewritten through the cumulative basis
      Qt_j(x) = sum_{k>=j} B_k(x) = Q((x - t_j)/h),   B_k = Qt_k - Qt_{k+1}
  where Q is a bounded [0,1] C^2 smoothstep.  Then
      sum_k W_k B_k(x) = sum_{j=0..8} (W_j - W_{j-1}) Qt_j(x)
  so each KAN layer becomes ONE dense matmul over 10 features per input
  (silu + 9 bounded Qt values).  6*Qt_j is computed on the vector engines in
  fp32 as a 3rd finite difference of capped relu-cubes
      phi_r = min(relu(2.5x + 5.5 - r), 11 - r)^3
      6*Qt_j = phi_j - 3 phi_{j+1} + 3 phi_{j+2} - phi_{j+3}
  (one fused custom DVE op per phi_r), and only the bounded results are
  rounded to fp32r (11-bit mantissa) for the full-rate fp32r matmul.
  The 1/6 and the weight differencing are folded into the weights on host
  in float64.  End-to-end error vs the fp32 reference: ~6e-4 L2.
"""

import numpy as np
from math import comb
from contextlib import ExitStack

# ---------------------------------------------------------------- constants
K = 10
B = 8192
CORES = 8
BC = B // CORES            # 1024 batch rows per core
IN0 = 2 * K                # 20
HID = 1024
OUT = 1024
NJ = 9                     # Qt features per input
NSEC = 1 + NJ              # silu + 9 Qt
NR = 12                    # relu-cube shifts r = 0..11
# layer-0 c-layout: 10 sections of 32 rows (20 used, partition starts must be
# 32-aligned) -> 320 rows (3 sbuf tiles)
C0_ROWS = 32 * NSEC        # 320
# layer-1 c-layout: c = i_tile*10 + s, 8 i_tiles -> 80 c-tiles of 128
C1_TILES = 8 * NSEC        # 80

_f32 = np.float32


def _round_f32r(a):
    """Round fp32 -> fp32r (11-bit mantissa, round-to-nearest-even)."""
    a = np.ascontiguousarray(a, np.float32)
    u = a.view(np.uint32).astype(np.uint64)
    drop = np.uint64(12)
    one = np.uint64(1)
    half = np.uint64(1 << 11)
    mask = ~np.uint64((1 << 12) - 1)
    r = (u + half - one + ((u >> drop) & one)) & mask
    return r.astype(np.uint32).view(np.float32)


def _poly_roots_host(x):
    """Exact copy of the reference poly_roots, forced onto CPU jax."""
    import jax
    cpu = jax.devices("cpu")[0]
    with jax.default_device(cpu):
        import jax.numpy as jnp
        xj = jax.device_put(np.asarray(x), cpu)
        coeffs = jax.lax.complex(xj[..., 0], xj[..., 1])
        b = coeffs.shape[0]
        norm = coeffs / coeffs[:, :1]
        c = -jnp.flip(norm[:, 1:], axis=-1)
        C = jnp.broadcast_to(jnp.eye(K, k=-1, dtype=coeffs.dtype), (b, K, K))
        C = C.at[:, :, -1].set(c)
        eigs = jnp.linalg.eigvals(C)
        out = jnp.stack([eigs.real, eigs.imag], axis=-1).reshape(b, 2 * K)
        return np.asarray(out.astype(jnp.float32))


def _fold_weights(bw, sw, sc):
    """[O,I] base + [O,I,8]*[O,I] spline weights -> [O, I, 10] folded:
    col 0 = base weight (silu feature), cols 1..9 = (W_j - W_{j-1})/6
    for the 6*Qt_j features."""
    W = sw.astype(np.float64) * sc.astype(np.float64)[..., None]        # [O,I,8]
    O, I = W.shape[:2]
    Wext = np.zeros((O, I, 10))
    Wext[:, :, 1:9] = W                                                  # Wext[m] = W_{m-1}
    wp = (Wext[:, :, 1:10] - Wext[:, :, 0:9]) / 6.0                      # [O,I,9]
    return np.concatenate([bw.astype(np.float64)[:, :, None], wp], axis=2)  # [O,I,10]


# ---------------------------------------------------------------- custom DVE op
_CAPCUBE = None


def _get_capcube():
    """out = min(relu(in0*imm2 + s0), s1)^3  — one DVE pass."""
    global _CAPCUBE
    if _CAPCUBE is not None:
        return _CAPCUBE
    from concourse.dve_spec import Spec, Src0, C0, C1, C2, minn, relu, sq, lower
    from concourse import dve_ops
    from concourse.dve_uop import DveOpSpec

    name = "KAN_CAPCUBE_ANT"
    if name in dve_ops._SUB_OPCODE_FOR_NAME:
        _CAPCUBE = next(op for op in dve_ops.OPS if op.name == name)
        return _CAPCUBE

    def _ref(in0, in1, s0, s1, imm2):
        m = np.minimum(np.maximum(in0 * imm2 + s0, 0.0), s1)
        return (m * m * m).astype(np.float32)

    m = minn(relu(Src0 * C2 + C0), C1)
    spec = Spec(body=sq(m) * m, reference=_ref)
    shas = {}
    for ver in ("v3", "v4"):
        try:
            s = DveOpSpec(name=name, opcode=1, uops=lower(spec, ver=ver), rd1_en=False)
            shas[ver] = s.sha(ver)
        except Exception:
            pass
    op = dve_ops.DveOp(name, spec, subdim=False, uops_sha=shas)
    dve_ops.OPS.append(op)
    dve_ops.CUSTOM_DVE_SPECS[name] = spec
    dve_ops._SUB_OPCODE_FOR_NAME[name] = dve_ops._CUSTOM_DVE_ROW_BASE + len(dve_ops.OPS) - 1
    _CAPCUBE = op
    return op


# ---------------------------------------------------------------- bass program
_PROGRAMS = {}

# phi shifts computed via ACT(relu) + DVE(min) + GPSIMD(sq, cube) instead of
# the fused DVE custom op, to balance engine load (DVE is the bottleneck)
_MOVED_R = ()


def build_program(iters=1):
    """Build (and cache) the compiled per-core Bass program.  iters>1 wraps
    the whole body in a device-side loop (for timing: amortizes the ~100 ms
    axon tunnel round-trip over many kernel executions)."""
    if iters in _PROGRAMS:
        return _PROGRAMS[iters]

    import concourse.bacc as bacc
    import concourse.bass as bass
    import concourse.tile as tile
    import concourse.mybir as mybir

    F32 = mybir.dt.float32
    F32R = mybir.dt.float32r
    AFT = mybir.ActivationFunctionType
    ALU = mybir.AluOpType
    CAPCUBE = _get_capcube()

    nc = bacc.Bacc("TRN2", target_bir_lowering=False, debug=False)

    rt_d = nc.dram_tensor("rt", [IN0, BC], F32, kind="ExternalInput")
    w0_d = nc.dram_tensor("w0t", [C0_ROWS, HID], F32R, kind="ExternalInput")
    w1_d = nc.dram_tensor("w1t", [C1_TILES, 128, OUT], F32R, kind="ExternalInput")
    out_d = nc.dram_tensor("out", [OUT, BC], F32, kind="ExternalOutput")

    with tile.TileContext(nc) as tc:
        with ExitStack() as ctx:
            const = ctx.enter_context(tc.tile_pool(name="const", bufs=1))
            f0p = ctx.enter_context(tc.tile_pool(name="f0p", bufs=1))
            tmp0p = ctx.enter_context(tc.tile_pool(name="tmp0p", bufs=2))
            hp = ctx.enter_context(tc.tile_pool(name="hp", bufs=8))
            phip = ctx.enter_context(tc.tile_pool(name="phip", bufs=1))
            chp = ctx.enter_context(tc.tile_pool(name="chp", bufs=4))
            qtp = ctx.enter_context(tc.tile_pool(name="qtp", bufs=2))
            silp = ctx.enter_context(tc.tile_pool(name="silp", bufs=3))
            w1p = ctx.enter_context(tc.tile_pool(name="w1p", bufs=4))
            osp = ctx.enter_context(tc.tile_pool(name="osp", bufs=4))
            psp = ctx.enter_context(tc.tile_pool(name="psp", bufs=8, space="PSUM"))

            loop_cm = tc.For_i(0, iters, 1) if iters > 1 else None
            if loop_cm is not None:
                loop_cm.__enter__()

            # ---------------- layer 0 ----------------
            rt_sb = const.tile([IN0, BC], F32)
            nc.sync.dma_start(rt_sb[:], rt_d.ap())

            f0 = [
                f0p.tile([128, BC], F32R, name="f0a"),
                f0p.tile([128, BC], F32R, name="f0b"),
                f0p.tile([64, BC], F32R, name="f0c"),
            ]
            # zero-fill (pad rows must be finite): DVE memset an f32 tile,
            # ACT-copy into the f32r tiles (activation producers round to f32r)
            zt = const.tile([128, BC], F32, name="zt")
            nc.vector.memset(zt[:], 0.0)
            for t in f0:
                p = t.shape[0]
                nc.scalar.copy(t[:], zt[0:p, :])

            # silu section (s=0) -> rows 0..19 of f0[0]
            nc.scalar.activation(f0[0][0:IN0, :], rt_sb[:], AFT.Silu)
            # Qt sections (s=1..9 at row 32*s), b in halves to bound phi size
            for bh in range(2):
                bs = slice(bh * 512, (bh + 1) * 512)
                phi0 = phip.tile([IN0, NR * 512], F32, name="phi")
                for r in range(NR):
                    nc.vector._custom_dve(
                        CAPCUBE,
                        out=phi0[:, r * 512:(r + 1) * 512],
                        in0=rt_sb[:, bs],
                        s0=float(5.5 - r),
                        s1=float(11 - r),
                        imm2=2.5,
                    )
                for j in range(NJ):
                    # qt6 = (phi_j - phi_{j+3}) + 3*(phi_{j+2} - phi_{j+1})
                    a = tmp0p.tile([IN0, 512], F32, name="l0a")
                    nc.gpsimd.tensor_sub(
                        a[:], phi0[:, j * 512:(j + 1) * 512],
                        phi0[:, (j + 3) * 512:(j + 4) * 512])
                    b = tmp0p.tile([IN0, 512], F32, name="l0b")
                    nc.vector.tensor_sub(
                        b[:], phi0[:, (j + 2) * 512:(j + 3) * 512],
                        phi0[:, (j + 1) * 512:(j + 2) * 512])
                    row = 32 * (1 + j)
                    ti, off = row // 128, row % 128
                    nc.vector.scalar_tensor_tensor(
                        f0[ti][off:off + IN0, bs],
                        b[:], 3.0, a[:], ALU.mult, ALU.add)

            w0 = [
                const.tile([128, HID], F32R, name="w0a"),
                const.tile([128, HID], F32R, name="w0b"),
                const.tile([64, HID], F32R, name="w0c"),
            ]
            nc.sync.dma_start(w0[0][:], w0_d.ap()[0:128, :])
            nc.sync.dma_start(w0[1][:], w0_d.ap()[128:256, :])
            nc.sync.dma_start(w0[2][:], w0_d.ap()[256:C0_ROWS, :])

            hT = [hp.tile([128, BC], F32, name="hT") for _ in range(8)]
            for bh in range(2):
                bsl = slice(bh * 512, (bh + 1) * 512)
                for o in range(8):
                    ps = psp.tile([128, 512], F32, name="ps")
                    for t in range(3):
                        nc.tensor.matmul(
                            ps[:], w0[t][:, o * 128:(o + 1) * 128], f0[t][:, bsl],
                            start=(t == 0), stop=(t == 2))
                    nc.scalar.copy(hT[o][:, bsl], ps[:])

            # ---------------- layer 1 ----------------
            for ch in range(2):
                bsl = slice(ch * 512, (ch + 1) * 512)
                pss = [psp.tile([128, 512], F32, name="ps") for _ in range(8)]
                for it in range(8):
                    x = hT[it][:, bsl]
                    phi = phip.tile([128, NR * 512], F32, name="phi")
                    for r in range(NR):
                        nc.vector._custom_dve(
                            CAPCUBE,
                            out=phi[:, r * 512:(r + 1) * 512],
                            in0=x,
                            s0=float(5.5 - r),
                            s1=float(11 - r),
                            imm2=2.5,
                        )
                    sil = silp.tile([128, 512], F32R, name="sil")
                    nc.scalar.activation(sil[:], x, AFT.Silu)
                    qt = qtp.tile([128, NJ * 512], F32R, name="qt")
                    for g in range(3):  # j-groups {0-2},{3-5},{6-8}
                        j0 = 3 * g
                        gw = 3 * 512
                        # qt6_j = (phi_j - phi_{j+3}) + 3*(phi_{j+2} - phi_{j+1})
                        a1 = chp.tile([128, gw], F32, name="ch")
                        nc.gpsimd.tensor_sub(
                            a1[:], phi[:, j0 * 512:j0 * 512 + gw],
                            phi[:, (j0 + 3) * 512:(j0 + 3) * 512 + gw])
                        a2 = chp.tile([128, gw], F32, name="ch")
                        nc.vector.tensor_sub(
                            a2[:], phi[:, (j0 + 2) * 512:(j0 + 2) * 512 + gw],
                            phi[:, (j0 + 1) * 512:(j0 + 1) * 512 + gw])
                        nc.vector.scalar_tensor_tensor(
                            qt[:, j0 * 512:j0 * 512 + gw],
                            a2[:], 3.0, a1[:], ALU.mult, ALU.add)
                    for s in range(NSEC):
                        c = it * NSEC + s
                        w = w1p.tile([128, OUT], F32R, name="w1")
                        nc.sync.dma_start(w[:], w1_d.ap()[c, :, :])
                        F = sil[:] if s == 0 else qt[:, (s - 1) * 512:s * 512]
                        for o in range(8):
                            nc.tensor.matmul(
                                pss[o][:], w[:, o * 128:(o + 1) * 128], F,
                                start=(c == 0), stop=(c == C1_TILES - 1))
                for o in range(8):
                    st = osp.tile([128, 512], F32, name="ost")
                    nc.scalar.copy(st[:], pss[o][:])
                    nc.sync.dma_start(out_d.ap()[o * 128:(o + 1) * 128, bsl], st[:])

            if loop_cm is not None:
                loop_cm.__exit__(None, None, None)

    nc.compile()
    _PROGRAMS[iters] = nc
    return nc


# ---------------------------------------------------------------- host driver
_INPUT_CACHE = {}


def prepare_inputs(x, bw0, sw0, sc0, bw1, sw1, sc1):
    """Host-side prep: roots + folded/pre-rounded weights + per-core in_maps."""
    roots = _poly_roots_host(np.asarray(x, np.float32))          # [B, 20]
    rootsT = np.ascontiguousarray(roots.T)                        # [20, B]

    W0 = _fold_weights(bw0, sw0, sc0)                             # [1024, 20, 10]
    W1 = _fold_weights(bw1, sw1, sc1)                             # [1024, 1024, 10]

    # layer-0 DRAM layout [320, 1024]: row 32*s + i  -> W0[o, i, s] (pads 0)
    w0t = np.zeros((C0_ROWS, HID), np.float64)
    for s in range(NSEC):
        w0t[32 * s:32 * s + IN0, :] = W0[:, :, s].T               # [20, 1024]
    w0t = _round_f32r(w0t.astype(np.float32))

    # layer-1 DRAM layout [80, 128, 1024]: (c = it*10 + s, p, o) -> W1[o, it*128+p, s]
    w1t = np.empty((C1_TILES, 128, OUT), np.float32)
    for it in range(8):
        blk = W1[:, it * 128:(it + 1) * 128, :]                   # [O, 128, 10]
        for s in range(NSEC):
            w1t[it * NSEC + s] = blk[:, :, s].T.astype(np.float32)
    w1t = _round_f32r(w1t)

    in_maps = []
    for c in range(CORES):
        in_maps.append({
            "rt": np.ascontiguousarray(rootsT[:, c * BC:(c + 1) * BC]),
            "w0t": w0t,
            "w1t": w1t,
        })
    return in_maps


def assemble_output(results):
    """Per-core [OUT, BC] (o, b) outputs -> full [B, OUT]."""
    return np.ascontiguousarray(
        np.concatenate([np.asarray(r["out"]).T for r in results], axis=0)
    ).astype(np.float32)


def kernel(x, bw0, sw0, sc0, bw1, sw1, sc1):
    from concourse.bass_utils import run_bass_kernel_spmd
    nc = build_program()
    in_maps = prepare_inputs(x, bw0, sw0, sc0, bw1, sw1, sc1)
    res = run_bass_kernel_spmd(nc, in_maps, list(range(CORES)))
    return assemble_output(res.results)
